# revision 19
# baseline (speedup 1.0000x reference)
"""Bass/Tile TRN2 kernel for nn_BasicRNN: out = scan(tanh(x@Wx + h@Wh) + h) @ Wout.

Data-parallel over batch across 8 NeuronCores (32 rows/core), recurrence
sequential in time on each core. No collectives; host gathers shards.

Numerics: the recurrence amplifies per-step perturbations ~70x, so plain
16-bit matmuls lose ~5-16% by t=256. The kernel therefore:
  - runs the recurrence in increment form (M_t = M_{t-1} + u_{t-1}@Wh,
    P_t = xproj_t + M_t, u_t = tanh(P_t), h_t = h_{t-1} + u_t) with fp32
    accumulators M and h, so only the bounded u passes through low precision;
  - splits matmul operands into hi+lo pairs: u@Wh uses u_hi|u_lo vs fp16
    Wh_hi plus u_hi vs Wh_lo, xproj splits both x and Wx (fp16).
  - Wh_lo is stored as fp8e4m3 scaled by 2^13 (LDWEIGHTS is the whmm
    bottleneck; fp8 FWL loads ~2x faster than fp16). To keep the PSUM
    domain consistent, ALL recurrence-side weights (Wx/Wh/b) carry the
    2^13 scale (exact power-of-2 in fp16), and the tanh activations
    descale for free via their scale argument. Wout_lo is dropped
    (emulation-verified negligible).
  Measured on hw: ~1.5e-3 final relative error vs the fp32 reference
  (gate 2e-2; full-fp16-pair scheme: 2.7e-4; plain bf16: 1.6e-1;
  dropping Wh_lo entirely: 3.8e-2 - the lo term must be applied every
  step, even lag-4 batched application fails at 2.1e-2).

Layout: h/u/M/P kept TRANSPOSED (hidden on partitions, packed [128,(c,b)])
so the serial chain needs no transposes. M = h@Wh lives in one persistent
PSUM bank that every step's matmuls accumulate into (hi|lo half-columns),
so the per-step chain is just two adds + tanh. xproj for a group of G steps
is batched into group PSUM banks ahead of time and staged to SBUF; xproj /
outproj matmuls are chopped into gap-sized thunks and interleaved between
steps as PE filler work, sized to the ~0.7us pointwise tail of each step.
"""

import sys

sys.path.insert(0, "/opt/trn_rl_repo")

from collections import deque

import numpy as np

import concourse.bass as bass  # noqa: F401
import concourse.tile as tile
from concourse import bacc, mybir
from concourse.bass_utils import run_bass_kernel_spmd

FP = mybir.dt.float32
F16 = mybir.dt.float16
F8 = mybir.dt.float8e4
TANH = mybir.ActivationFunctionType.Tanh

B, D, T, H, OUT = 256, 256, 256, 512, 256
NCORES = 8
BC = B // NCORES  # 32 batch rows per core
P = 128
DC = D // P  # 2 d-chunks
HC = H // P  # 4 h-chunks

# Uniform scale applied to all recurrence-side weights (Wx/Wh/b): P-domain
# PSUM values are S*P, descaled for free via tanh's scale argument. Lets
# Wh_lo live in fp8e4m3 (S*Wh_lo spans ~[2^-9, 0.25] - in e4m3 range)
# whose LDWEIGHTS is ~2x faster than fp16, halving the lo-pass PE cost.
SCALE_BITS = 13
S = float(2 ** SCALE_BITS)
INV_S = float(2.0 ** -SCALE_BITS)


def build(T_=T, G=8, reps=1, fill_per_step=3, h_gpsimd=False,
          lo_fp8=True, op_lo=False, no_stage=False, hist_direct=False,
          parts=("whmm", "pointwise", "xproj", "outproj")):
    parts = set(parts)
    NG = T_ // G
    GB = G * BC            # (t, b) free width of one group = 256
    TPM = P // BC          # timesteps per outproj M-chunk = 4
    MCG = GB // P          # outproj M-chunks per group = 2
    UW = HC * 2 * BC       # u2 / step-psum width (k, hi|lo, b) = 256
    HB = HC * BC           # packed h width (c, b) = 128
    assert T_ % G == 0 and GB <= 512

    nc = bacc.Bacc("TRN2", target_bir_lowering=False, debug=False, num_devices=NCORES)

    x_d = nc.declare_dram_parameter("x", [BC, D, T_], FP, isOutput=False)
    wx_d = nc.declare_dram_parameter("Wx", [D, H], FP, isOutput=False)
    wh_d = nc.declare_dram_parameter("Wh", [H, H], FP, isOutput=False)
    b_d = nc.declare_dram_parameter("b", [H], FP, isOutput=False)
    wo_d = nc.declare_dram_parameter("Wout", [H, OUT], FP, isOutput=False)
    bo_d = nc.declare_dram_parameter("bout", [OUT], FP, isOutput=False)
    init_d = nc.declare_dram_parameter("init_state", [1, H], FP, isOutput=False)
    out_d = nc.declare_dram_parameter("out", [BC, T_, OUT], FP, isOutput=True)

    with tile.TileContext(nc) as tc:
        with (
            tc.tile_pool(name="const", bufs=1) as const,
            tc.tile_pool(name="xbuf", bufs=1) as xbuf,
            tc.tile_pool(name="xg", bufs=3) as xg_pool,
            tc.tile_pool(name="h0p", bufs=1) as h0p,
            tc.tile_pool(name="hist", bufs=3) as hist_pool,
            tc.tile_pool(name="upool", bufs=3) as upool,
            tc.tile_pool(name="ufpool", bufs=3) as ufpool,
            tc.tile_pool(name="hfpool", bufs=3) as hfpool,
            tc.tile_pool(name="ppool", bufs=3) as ppool,
            tc.tile_pool(name="p1pool", bufs=3) as p1pool,
            tc.tile_pool(name="xps", bufs=2) as xps_pool,
            tc.tile_pool(name="stg", bufs=4) as stg_pool,
            tc.tile_pool(name="xpp", bufs=2, space="PSUM") as xp_psum,
            tc.tile_pool(name="macc", bufs=1, space="PSUM") as macc_pool,
            tc.tile_pool(name="opp", bufs=2, space="PSUM") as op_psum,
        ):
            # ---------------- one-time prologue: weights + x ----------------
            def load_split(dram_ap, rows, cols, nm, scale=1.0, lo_dt=F16):
                """DMA fp32 weights, make (scaled) fp16 hi + lo tiles."""
                f = const.tile([rows, cols], FP, name=f"{nm}f")
                hi = const.tile([rows, cols], F16, name=f"{nm}h")
                lo = const.tile([rows, cols], lo_dt, name=f"{nm}l")
                nc.sync.dma_start(out=f[:, :], in_=dram_ap)
                if scale != 1.0:
                    fs = const.tile([rows, cols], FP, name=f"{nm}fs")
                    nc.vector.tensor_scalar_mul(fs[:, :], f[:, :], scale)
                    f = fs
                nc.vector.tensor_copy(hi[:, :], f[:, :])
                nc.vector.tensor_sub(lo[:, :], f[:, :], hi[:, :])
                return hi, lo

            wh_hi, wh_lo = [], []
            for k in range(HC):
                hi, lo = load_split(wh_d[k * P:(k + 1) * P, :], P, H, f"wh{k}",
                                    scale=S, lo_dt=F8 if lo_fp8 else F16)
                wh_hi.append(hi); wh_lo.append(lo)
            wx_hi, wx_lo = [], []
            for d in range(DC):
                hi, lo = load_split(wx_d[d * P:(d + 1) * P, :], P, H, f"wx{d}",
                                    scale=S)
                wx_hi.append(hi); wx_lo.append(lo)
            wo_hi, wo_lo = [], []
            for k in range(HC):
                hi, lo = load_split(wo_d[k * P:(k + 1) * P, :], P, OUT, f"wo{k}")
                wo_hi.append(hi); wo_lo.append(lo)
            b_hi, b_lo = load_split(b_d[:].rearrange("(o h) -> o h", o=1), 1, H, "b",
                                    scale=S)
            bo_hi, bo_lo = load_split(bo_d[:].rearrange("(o h) -> o h", o=1), 1, OUT, "bo")

            ones = const.tile([1, 512], F16, name="ones")
            nc.vector.memset(ones[:, :], 1.0)
            zrow = const.tile([1, P], F16, name="zrow")
            nc.vector.memset(zrow[:, :], 0.0)

            init_sb = const.tile([P, HC], FP, name="initsb")
            nc.sync.dma_start(
                out=init_sb[:, :], in_=init_d[0, :].rearrange("(c p) -> p c", p=P)
            )

            # x resident in SBUF fp32: per d-chunk, free=(b,t) so the DMA moves
            # 1KB-contiguous t-rows; loaded in quarters so compute starts early.
            x_f = [xbuf.tile([P, BC * T_], FP, name=f"xf{d}") for d in range(DC)]
            NQ = 4 if T_ % 4 == 0 else 1
            TQ = T_ // NQ
            for q in range(NQ):
                for d in range(DC):
                    dst = x_f[d].rearrange("p (b t) -> p b t", b=BC)[:, :, q * TQ:(q + 1) * TQ]
                    src = x_d[:, d * P:(d + 1) * P, q * TQ:(q + 1) * TQ].rearrange("b d t -> d b t")
                    nc.sync.dma_start(out=dst, in_=src)

            # ---------------- per-run body (repeatable for timing) ----------
            def body():
                # h0 = broadcast(init_state); fp32 + fp16 hi/lo (u_{-1} seed)
                h0_f = h0p.tile([P, HB], FP, name="h0f")
                nc.vector.memset(h0_f[:, :], 0.0)
                for c in range(HC):
                    nc.vector.tensor_scalar_add(
                        h0_f[:, c * BC:(c + 1) * BC],
                        h0_f[:, c * BC:(c + 1) * BC],
                        init_sb[:, c:c + 1],
                    )
                u2_0 = h0p.tile([P, UW], F16, name="u20")
                u2_0v = u2_0.rearrange("p (k two b) -> p k two b", k=HC, two=2)
                h0_3 = h0_f.rearrange("p (c b) -> p c b", c=HC)
                nc.vector.tensor_copy(u2_0v[:, :, 0, :], h0_3)
                nc.vector.tensor_sub(u2_0v[:, :, 1, :], h0_3, u2_0v[:, :, 0, :])
                # persistent M accumulator in PSUM: macc = h_{t-1} @ Wh in
                # hi|lo half-columns. Cleared by step 0's first MM running
                # with start=True (whole-bank has_written clear; every column
                # is written by some step-0 MM, overwrite-where-clear).
                macc = macc_pool.tile([P, UW], FP, name="macc")
                if "whmm" not in parts:
                    nc.tensor.matmul(
                        out=macc[:, :], lhsT=zrow[0:1, :], rhs=ones[0:1, 0:UW],
                        start=True, stop=False, skip_group_check=True,
                    )
                macc3 = macc.rearrange("p (m two b) -> p m two b", m=HC, two=2)

                xp_tiles = {}
                xps_tiles = {}
                hist_tiles = {}
                fillers = deque()
                BANK_F32 = 512

                def xproj_thunks(g):
                    xp = xp_psum.tile([P, HC * GB], FP, name=f"xp{g}", tag="xp")
                    xp_tiles[g] = xp
                    xgh = [xg_pool.tile([P, GB], F16, name=f"xgh{g}_{d}", tag=f"xgh{d}") for d in range(DC)]
                    xgl = [xg_pool.tile([P, GB], F16, name=f"xgl{g}_{d}", tag=f"xgl{d}") for d in range(DC)]
                    ths = []

                    def prep(g=g, xp=xp):
                        # per-group x slice -> fp16 hi/lo (DVE). No explicit
                        # bank zeroing: the first MM per 512-col PSUM bank
                        # runs with start=True, which clears the whole bank's
                        # has_written bits; later MMs overwrite-where-clear.
                        # hi/lo casts on GpSimd: it is otherwise idle, these
                        # are SBUF->SBUF, and their consumers (xproj MMs) pop
                        # 1-2 steps later - while DVE carries the serial chain.
                        for d in range(DC if "xproj" in parts else 0):
                            src = x_f[d].rearrange("p (b t) -> p t b", b=BC)[:, g * G:(g + 1) * G, :]
                            dsth = xgh[d].rearrange("p (t b) -> p t b", t=G)
                            dstl = xgl[d].rearrange("p (t b) -> p t b", t=G)
                            nc.gpsimd.tensor_copy(dsth, src)
                            nc.gpsimd.tensor_sub(dstl, src, dsth)
                        if "xproj" not in parts:
                            for bk in range((HC * GB) // BANK_F32):
                                nc.tensor.matmul(
                                    out=xp[:, bk * BANK_F32:(bk + 1) * BANK_F32],
                                    lhsT=zrow[0:1, :],
                                    rhs=ones[0:1, 0:BANK_F32],
                                    start=True, stop=False, skip_group_check=True,
                                )
                    ths.append(prep)

                    for m in range(HC if "xproj" in parts else 0):
                        def th_a(m=m, g=g, xp=xp):
                            out_ap = xp[:, m * GB:(m + 1) * GB]
                            # m even -> this MM is the first touching its bank
                            fst = (m * GB) % BANK_F32 == 0
                            for lhsT, rhs in (
                                (wx_hi[0], xgh[0]),
                                (wx_hi[0], xgl[0]),
                                (wx_lo[0], xgh[0]),
                            ):
                                nc.tensor.matmul(
                                    out=out_ap,
                                    lhsT=lhsT[:, m * P:(m + 1) * P],
                                    rhs=rhs[:, :],
                                    start=fst, stop=False, skip_group_check=True,
                                )
                                fst = False
                        def th_b(m=m, g=g, xp=xp):
                            out_ap = xp[:, m * GB:(m + 1) * GB]
                            for lhsT, rhs in (
                                (wx_hi[1], xgh[1]),
                                (wx_hi[1], xgl[1]),
                                (wx_lo[1], xgh[1]),
                            ):
                                nc.tensor.matmul(
                                    out=out_ap,
                                    lhsT=lhsT[:, m * P:(m + 1) * P],
                                    rhs=rhs[:, :],
                                    start=False, stop=False, skip_group_check=True,
                                )
                            for brow in (b_hi, b_lo):
                                nc.tensor.matmul(
                                    out=out_ap,
                                    lhsT=brow[0:1, m * P:(m + 1) * P],
                                    rhs=ones[0:1, 0:GB],
                                    start=False, stop=False, skip_group_check=True,
                                )
                        ths.append(th_a)
                        ths.append(th_b)

                    if "xproj" in parts:
                        if no_stage:
                            xps_tiles[g] = xp  # read P-domain input straight from PSUM
                        else:
                            xps = xps_pool.tile([P, HC * GB], FP, name=f"xps{g}", tag="xps")
                            xps_tiles[g] = xps
                            HW_ = (HC * GB) // 2
                            # stage on ScalarE: it is the least-loaded engine,
                            # sits closer to PSUM, and its FIFO has >1us of
                            # idle before the next tanh becomes ready - while
                            # DVE carries the serial p1/pt/sub chain.
                            ths.append(lambda: nc.scalar.copy(xps[:, 0:HW_], xp[:, 0:HW_]))
                            ths.append(lambda: nc.scalar.copy(xps[:, HW_:], xp[:, HW_:]))
                    return ths

                def outproj_thunks(g):
                    hist = hist_tiles[g]
                    ths = []
                    for mc in range(MCG):
                        ops = op_psum.tile([P, OUT], FP, name=f"op{g}_{mc}", tag="op")

                        def mm_half(ks, first, mc=mc, g=g, hist=hist, ops=ops):
                            fst = first
                            wo_parts = (wo_hi, wo_lo) if op_lo else (wo_hi,)
                            for k in ks:
                                # hist free layout is (c, t, b): for chunk k,
                                # M-chunk mc covers a contiguous 128-col run.
                                lhsT = hist[:, k * G * BC + mc * P: k * G * BC + (mc + 1) * P]
                                for wo in wo_parts:
                                    nc.tensor.matmul(
                                        out=ops[:, :], lhsT=lhsT, rhs=wo[k][:, :],
                                        start=fst, stop=False,
                                    )
                                    fst = False

                        def tail(mc=mc, g=g, ops=ops):
                            # bout as single fp16 rank-1 (lo term negligible:
                            # error <= 2^-11 * |bout| directly on the output)
                            nc.tensor.matmul(
                                out=ops[:, :], lhsT=ones[0:1, 0:P], rhs=bo_hi[0:1, :],
                                start=False, stop=True,
                            )
                            stg = stg_pool.tile([P, OUT], FP, name=f"st{g}_{mc}", tag="stg")
                            nc.scalar.copy(stg[:, :], ops[:, :])
                            t0 = g * G + mc * TPM
                            dst = out_d[:, t0:t0 + TPM, :].rearrange("b t o -> t b o")
                            nc.sync.dma_start(out=dst, in_=stg[:, :])

                        ths.append(lambda mm=mm_half: mm((0, 1), True))
                        ths.append(lambda mm=mm_half: mm((2, 3), False))
                        ths.append(tail)
                    return ths

                for th in xproj_thunks(0):
                    th()

                if hist_direct:
                    prev_f = h0_f.rearrange("p (c b) -> p c b", c=HC)
                else:
                    prev_f = h0_f[:, :]  # h_{t-1} fp32
                prev_u = u2_0           # u_{t-1} fp16 hi|lo packed [128,(k,2,b)]

                for t in range(T_):
                    g, tl = divmod(t, G)
                    if tl == 0:
                        while fillers:
                            fillers.popleft()()
                        hist_tiles[g] = hist_pool.tile(
                            [P, G * HB], F16, name=f"hist{g}", tag="hist"
                        )
                        if g + 1 < NG:
                            fillers.extend(xproj_thunks(g + 1))
                        if g >= 1 and "outproj" in parts:
                            fillers.extend(outproj_thunks(g - 1))

                    # ---- u_{t-1} @ Wh accumulated into persistent macc ----
                    # Wh_lo terms first (fp8 weights: ~2x faster LDWEIGHTS):
                    # they need only u_hi, so they start as soon as the
                    # previous step's first tanh lands. Step 0's first MM
                    # carries start=True to clear the macc bank for this rep.
                    for k in range(HC if "whmm" in parts else 0):
                        for m in range(HC):
                            nc.tensor.matmul(
                                out=macc[:, m * 2 * BC: m * 2 * BC + BC],
                                lhsT=wh_lo[k][:, m * P:(m + 1) * P],
                                rhs=prev_u[:, k * 2 * BC: k * 2 * BC + BC],
                                start=(t == 0 and k == 0 and m == 0),
                                stop=False, skip_group_check=True,
                            )
                    for k in range(HC if "whmm" in parts else 0):
                        for m in range(HC):
                            nc.tensor.matmul(
                                out=macc[:, m * 2 * BC:(m + 1) * 2 * BC],
                                lhsT=wh_hi[k][:, m * P:(m + 1) * P],
                                rhs=prev_u[:, k * 2 * BC:(k + 1) * 2 * BC],
                                start=False, stop=False, skip_group_check=True,
                            )

                    if "pointwise" not in parts:
                        if "spcast" in parts:
                            hdst = hist_tiles[g].rearrange(
                                "p (c t b) -> p c t b", c=HC, t=G
                            )[:, :, tl, :]
                            nc.vector.tensor_copy(hdst, macc3[:, :, 0, :])
                        for _ in range(fill_per_step):
                            if fillers:
                                fillers.popleft()()
                        continue

                    xps = xps_tiles.get(g)
                    if xps is None:  # probe mode: pointwise without xproj
                        if "dummy_xps" not in xps_tiles:
                            dx = h0p.tile([P, HC * GB], FP, name="dummyxps")
                            nc.vector.memset(dx[:, :], 0.0)
                            xps_tiles["dummy_xps"] = dx
                        xps = xps_tiles["dummy_xps"]
                    xp_slice = xps.rearrange("p (m t b) -> p m t b", m=HC, t=G)[:, :, tl, :]
                    p1 = p1pool.tile([P, HB], FP, name=f"p1_{t}", tag="p1")
                    p1v = p1.rearrange("p (c b) -> p c b", c=HC)
                    nc.vector.tensor_add(p1v, macc3[:, :, 0, :], xp_slice)
                    pt = ppool.tile([P, HB], FP, name=f"p{t}", tag="p")
                    ptv = pt.rearrange("p (c b) -> p c b", c=HC)
                    nc.vector.tensor_add(ptv, macc3[:, :, 1, :], p1v)

                    u2 = upool.tile([P, UW], F16, name=f"u{t}", tag="u")
                    u2v = u2.rearrange("p (k two b) -> p k two b", k=HC, two=2)
                    nc.scalar.activation(u2v[:, :, 0, :], ptv, TANH, scale=INV_S)
                    uf = ufpool.tile([P, HB], FP, name=f"uf{t}", tag="uf")
                    nc.scalar.activation(uf[:, :], pt[:, :], TANH, scale=INV_S)
                    nc.vector.tensor_sub(
                        u2v[:, :, 1, :],
                        uf.rearrange("p (c b) -> p c b", c=HC),
                        u2v[:, :, 0, :],
                    )

                    heng = nc.gpsimd if h_gpsimd else nc.vector
                    # hist free layout (c, t, b) so outproj weight slices are
                    # contiguous (BIR: weights AP must be single-free-dim)
                    hdst = hist_tiles[g].rearrange("p (c t b) -> p c t b", c=HC, t=G)[:, :, tl, :]
                    if hist_direct:
                        # h accumulates in fp16 directly in the hist slot
                        # (one op instead of add+copy; ~2x h rounding error)
                        heng.tensor_add(hdst, uf.rearrange("p (c b) -> p c b", c=HC), prev_f)
                        prev_f = hdst
                    else:
                        hf = hfpool.tile([P, HB], FP, name=f"hf{t}", tag="hf")
                        heng.tensor_add(hf[:, :], uf[:, :], prev_f)
                        heng.tensor_copy(hdst, hf.rearrange("p (c b) -> p c b", c=HC))
                        prev_f = hf[:, :]
                    prev_u = u2

                    for _ in range(fill_per_step):
                        if fillers:
                            fillers.popleft()()

                while fillers:
                    fillers.popleft()()
                if "outproj" in parts:
                    for th in outproj_thunks(NG - 1):
                        th()

            if reps > 4:
                with tc.For_i(0, reps, 1):
                    body()
            else:
                for _ in range(reps):
                    body()

    nc.compile()
    return nc


_NC_CACHE = {}


def _get_nc(T_=T, G=8, reps=1):
    key = (T_, G, reps)
    if key not in _NC_CACHE:
        _NC_CACHE[key] = build(T_, G, reps)
    return _NC_CACHE[key]


def run(inputs, T_=T, G=8, reps=1):
    nc = _get_nc(T_, G, reps)
    x = np.ascontiguousarray(np.asarray(inputs["x"], dtype=np.float32))
    shared = {
        k: np.ascontiguousarray(np.asarray(inputs[k], dtype=np.float32))
        for k in ("Wx", "Wh", "b", "Wout", "bout", "init_state")
    }
    core_ids = list(range(NCORES))
    in_maps = [{"x": x[c * BC:(c + 1) * BC], **shared} for c in core_ids]
    res = run_bass_kernel_spmd(nc, in_maps, core_ids)
    out = np.concatenate([res.results[c]["out"] for c in core_ids], axis=0)
    return out


def kernel(**inputs):
    return run(inputs)


if __name__ == "__main__":
    import time

    t0 = time.time()
    _get_nc()
    print(f"build: {time.time() - t0:.1f}s")

